# revision 14
# baseline (speedup 1.0000x reference)
"""Multi-head self-attention (B=8, S=2048, H=256, NH=8, HD=32) on 8 TRN2 cores.

Strategy: data-parallel over batch — each core computes full MHA for one
batch element; no collectives.

Per-core dataflow (matmuls bf16 in / fp32 PSUM accum):
  - host ships x^T (features on partitions) so no on-device transpose
  - attention runs qb(4) x g(2 head-groups) x kt(16) x half(2):
    per kt all 4 row-tiled scores matmuls (tile_position=(32j,0), K=32)
    adjacent so they overlap 4-way, into a 3-deep rotation of [128,1024]
    PSUM tiles, then one exp per half:
      - ACT steps: scalar ACTIVATE Exp (scale folded in)
      - DVE steps: Schraudolph bf16 exp — tensor_scalar mult+add to an
        int16 view of the bf16 eT tile (i16 = rne(s*A + B) IS the bf16
        bit pattern of ~exp(s*scale)); rowsum-normalization cancels the
        systematic part of the approx error per head
    Splitting exp across both engines breaks the single-engine ACT
    bottleneck; the PE runs at the 1.2GHz mid p-state throughout (the
    2.4GHz gate needs >3us gap-free matmul streaks this dataflow can't
    sustain), so scores/ctx cost ~427ns per 512-col matmul stream
  - ctx^T accumulated over kt with 2x column-tiled PE (tile_position
    (0,0)/(0,64)); stationary v blocks carry a ones column so each
    64-wide head slot yields [ctx_h(32) | rowsum(1)]
  - g-outer keeps only 2 ctx accumulator banks live -> scores get a
    3-tile rotation (6 banks) which decouples the scores->exp WAR chain
  - ALL other PSUM users (warmup, q/k/v projections, out-projection)
    borrow slots from the scores rotation pool, so the q/k/v projections
    pipeline INTO the first attention group instead of a serial head
    phase: t=0/t=2 strips of w_qkv and v st-chunks 0,1 run before the
    first scores; v st=2..15 and the t=1/t=3 strips interleave into
    (qb0,g0) kt iterations just ahead of their first use
  - per (qb,g): ctx PSUM evicted to bf16 staging in one [97,512] copy;
    rowsum rows gathered by tiny SBUF->SBUF DMAs into a packed [64,32]
    tile (reciprocal free-size 32), broadcast back via DRAM; the
    normalization multiplies run on the otherwise-idle GpSimd; all
    chain pieces are emitted deferred (during the next group) so DMA
    latency never stalls the in-order engine queues
  - out = ctxT^T @ w_out_perm + b_out as a tail phase via the same
    rotation pool (no pool-open barrier); evictions alternate ACT/DVE
"""
import numpy as np
import ml_dtypes

import bass_rust
import concourse.bass as bass
import concourse.mybir as mybir
import concourse.tile as tile
from concourse.bass_utils import run_bass_kernel_spmd

BF16 = mybir.dt.bfloat16
F32 = mybir.dt.float32
I16 = mybir.dt.int16
NPBF16 = ml_dtypes.bfloat16

B, S, H = 8, 2048, 256
NH, HD = 8, 32
SCALE = 1.0 / float(np.sqrt(HD))
N_CORES = 8

# Schraudolph bf16 exp constants: i16 = rne(s*A + B) viewed as bf16
# approximates exp(s*SCALE).  A = SCALE * 2^7 / ln2; B = 127*2^7 - c with
# c=1.5 calibrated for min global error (rne rounding confirmed on HW).
EXP_A = SCALE * 128.0 / float(np.log(2.0))
EXP_B = 16256.0 - 1.5

# ACT/DVE exp split: pattern over 32 steps, True -> ACT. 18/32 on ACT.
N_ACT_OF_32 = 18
ACT_PAT = [((i + 1) * N_ACT_OF_32) // 32 - (i * N_ACT_OF_32) // 32 == 1
           for i in range(32)]

# Set by a test harness to collect HW timing: {"trace": bool, "trace_cores": [...]}
TRACE_OPTS = {}
LAST_RESULT = None


def _legalize_sync_waits(nc):
    """The walrus build here rejects >1 sync wait per instruction, but Tile
    freely emits 2-3 (and the exit drain up to ~27).  Move excess waits onto
    same-engine NoOp carriers inserted immediately before the offending
    instruction — identical semantics (the engine blocks on each wait in
    program order)."""
    n = 0
    for f in nc.m.functions:
        for bb in f.blocks:
            insts = bb.instructions  # live list
            i = 0
            while i < len(insts):
                inst = insts[i]
                si = inst.sync_info
                if si is not None and len(si.on_wait) > 1:
                    waits = list(si.on_wait)
                    carriers = []
                    for w in waits[:-1]:
                        carriers.append(
                            mybir.InstNoOp(
                                name=f"{inst.name}-w{n}",
                                sync_info=mybir.SyncInfo(on_wait=[w], on_update=[]),
                                bass_nofuse=True,
                                engine=inst.engine,
                            )
                        )
                        n += 1
                    inst.sync_info = bass_rust.SyncInfo(
                        on_wait=waits[-1:], on_update=list(si.on_update)
                    )
                    insts[i:i] = carriers
                    i += len(carriers)
                i += 1
    return n


def _build_nc(legalize=True):
    nc = bass.Bass()
    xt = nc.dram_tensor("xt", [128, 2 * S], BF16, kind="ExternalInput")
    wqk = nc.dram_tensor("wqk", [128, 2 * 512], BF16, kind="ExternalInput")
    bv = nc.dram_tensor("bv", [1, 264], BF16, kind="ExternalInput")
    wv = nc.dram_tensor("wv", [128, 2 * 264], BF16, kind="ExternalInput")
    wo = nc.dram_tensor("wo", [128, 4 * 256], BF16, kind="ExternalInput")
    bqkc = nc.dram_tensor("bqkc", [128, 4], F32, kind="ExternalInput")
    ones = nc.dram_tensor("ones", [1, 512], BF16, kind="ExternalInput")
    zrow = nc.dram_tensor("zrow", [2, 2048], BF16, kind="ExternalInput")
    out = nc.dram_tensor("out", [S, H], F32, kind="ExternalOutput")
    # scratch for the rowsum-reciprocal broadcast (SBUF APs cannot have a
    # zero partition step, DRAM APs can): one row per (qb, g, h, side)
    rscr2 = nc.dram_tensor("rscr2", [32, 512], BF16)

    EXP = mybir.ActivationFunctionType.Exp

    with tile.TileContext(nc) as tc:
        with (
            tc.tile_pool(name="const", bufs=1) as const,
            tc.tile_pool(name="ev", bufs=6) as ev,
            tc.tile_pool(name="stgp", bufs=4) as stgp,
            tc.tile_pool(name="rcbp", bufs=4) as rcbp,
            tc.tile_pool(name="etp", bufs=8) as etp,
            tc.tile_pool(name="scp", bufs=3, space="PSUM") as scp,
            tc.tile_pool(name="cxp", bufs=2, space="PSUM") as cxp,
        ):
            # critical-path inputs first: wqk + xt gate the first projections
            wqk_sb = const.tile([128, 2 * 512], BF16, tag="wqk")
            nc.sync.dma_start(out=wqk_sb, in_=wqk[:, :])
            xt_sb = const.tile([128, 2 * S], BF16, tag="xt")
            nc.sync.dma_start(out=xt_sb, in_=xt[:, :])
            wv_sb = const.tile([128, 2 * 264], BF16, tag="wv")
            nc.sync.dma_start(out=wv_sb, in_=wv[:, :])
            bv_sb = const.tile([1, 264], BF16, tag="bv")
            nc.sync.dma_start(out=bv_sb, in_=bv[:, :])
            ones1_sb = const.tile([1, 128], BF16, tag="ones1")
            nc.sync.dma_start(out=ones1_sb, in_=ones[0:1, 0:128])
            bqkc_sb = const.tile([128, 4], F32, tag="bqkc")
            nc.sync.dma_start(out=bqkc_sb, in_=bqkc[:, :])
            wo_sb = const.tile([128, 4 * 256], BF16, tag="wo")
            nc.sync.dma_start(out=wo_sb, in_=wo[:, :])

            qT_sb = const.tile([128, 2 * S], BF16, tag="qT")
            kT_sb = const.tile([128, 2 * S], BF16, tag="kT")
            v_sb = const.tile([128, 16 * 264], BF16, tag="v")
            ctxT_sb = [
                const.tile([128, S], BF16, tag=f"ctxT{k}", name=f"ctxT{k}")
                for k in range(4)
            ]

            # ---- projection chunk emitters (all PSUM via the scp rotation) --
            def emit_qk_chunk(t, nb):
                # q/k strip tile t, s-block nb -> qT/kT with bias folded in
                ps = scp.tile([128, 1024], F32, tag="sc", name=f"qk_{t}_{nb}")
                for ks in range(2):
                    nc.tensor.matmul(
                        out=ps[:, 0:512],
                        lhsT=wqk_sb[:, ks * 512 + t * 128 : ks * 512 + t * 128 + 128],
                        rhs=xt_sb[:, ks * S + nb * 512 : ks * S + nb * 512 + 512],
                        start=(ks == 0), stop=(ks == 1),
                    )
                dst = (qT_sb if t < 2 else kT_sb)[
                    :, (t % 2) * S + nb * 512 : (t % 2) * S + nb * 512 + 512
                ]
                nc.vector.tensor_scalar_add(
                    out=dst, in0=ps[:, 0:512], scalar1=bqkc_sb[:, t : t + 1]
                )

            def emit_v_chunk(st):
                # v st-chunk (natural layout, padded 64-wide head slots,
                # ones column at j=32 for rowsums; bias row plants the ones)
                ps = scp.tile([128, 1024], F32, tag="sc", name=f"v_{st}")
                for ks in range(2):
                    nc.tensor.matmul(
                        out=ps[:, 0:264],
                        lhsT=xt_sb[:, ks * S + st * 128 : ks * S + st * 128 + 128],
                        rhs=wv_sb[:, ks * 264 : ks * 264 + 264],
                        start=(ks == 0), stop=False,
                    )
                nc.tensor.matmul(
                    out=ps[:, 0:264],
                    lhsT=ones1_sb[0:1, 0:128],
                    rhs=bv_sb[0:1, 0:264],
                    start=False, stop=True,
                )
                dst = v_sb[:, st * 264 : st * 264 + 264]
                if st % 2 == 0:
                    nc.scalar.copy(out=dst, in_=ps[:, 0:264])
                else:
                    nc.vector.tensor_copy(out=dst, in_=ps[:, 0:264])

            # ---- warmup + the minimum projections to start attention ----
            warm_sb = const.tile([128, 512], BF16, tag="warm")
            nc.vector.memset(warm_sb, 0.0)
            warm_ps = scp.tile([128, 1024], F32, tag="sc", name="warm")
            for i in range(4):
                nc.tensor.matmul(
                    out=warm_ps[:, 0:512], lhsT=warm_sb[:, 0:128],
                    rhs=warm_sb[:, :], start=(i == 0), stop=(i == 3),
                )
            for nb in range(4):
                emit_qk_chunk(0, nb)   # q heads 0-3
            for nb in range(4):
                emit_qk_chunk(2, nb)   # k heads 0-3
            emit_v_chunk(0)
            emit_v_chunk(1)
            # remaining projections interleave into (qb0, g0) below
            deferred_proj = [("v", st) for st in range(2, 16)]
            late_qk = [(1, nb) for nb in range(4)] + [(3, nb) for nb in range(4)]

            # ctxT zero-init: rows 32:64 / 96:128 are contracted by the
            # output matmul against zeroed w_out rows — clear them so stale
            # NaN patterns can't poison the accumulation (row 32 of tile 0 is
            # all-ones: paired with w_out_perm row 32 = b_out it adds the
            # output bias for free)
            for k in range(4):
                if k == 0:
                    nc.sync.dma_start(out=ctxT_sb[0][32:33, :], in_=zrow[1:2, :])
                    nc.sync.dma_start(
                        out=ctxT_sb[0][33:64, :],
                        in_=zrow[0:1, :].to_broadcast((31, S)),
                    )
                else:
                    nc.sync.dma_start(
                        out=ctxT_sb[k][32:64, :],
                        in_=zrow[0:1, :].to_broadcast((32, S)),
                    )
                nc.sync.dma_start(
                    out=ctxT_sb[k][96:128, :],
                    in_=zrow[0:1, :].to_broadcast((32, S)),
                )

            # ---- attention ----
            stg_tiles = {}   # (qb, g, h) -> stg tile
            rsg_tiles = {}   # (qb, g) -> packed bf16 rowsum gather tile

            def emit_ctx(qb, g, kt, ctx_t, eTs):
                # ctx accumulation for (g, kt): 4 col-mode matmuls
                for h in range(2):
                    cps = ctx_t[h]
                    eT = eTs[h]
                    vc = kt * 264 + (4 * g + 2 * h) * 33
                    nc.tensor.matmul(
                        out=cps[0:33, :],
                        lhsT=v_sb[:, vc : vc + 33],
                        rhs=eT[:, 0:512],
                        start=(kt == 0), stop=(kt == 15),
                        tile_position=(0, 0), skip_group_check=True,
                    )
                    nc.tensor.matmul(
                        out=cps[64:97, :],
                        lhsT=v_sb[:, vc + 33 : vc + 66],
                        rhs=eT[:, 512:1024],
                        start=(kt == 0), stop=(kt == 15),
                        tile_position=(0, 64), skip_group_check=True,
                    )

            def emit_recip(qb, g):
                # rowsum reciprocal for the 4 (h,side) rows of (qb,g),
                # packed [64,32] (free size 32) so the DVE cost is tiny;
                # result lands in rscr2 rows for the broadcast DMAs
                rsgb = rsg_tiles.pop((qb, g))
                rsgf = ev.tile([64, 32], F32, tag="rsgf")
                nc.vector.tensor_copy(out=rsgf, in_=rsgb)
                rsr = ev.tile([64, 32], F32, tag="rsr")
                nc.vector.reciprocal(out=rsr, in_=rsgf)
                rsb = ev.tile([64, 32], BF16, tag="rsb")
                nc.vector.tensor_copy(out=rsb, in_=rsr)
                r0 = qb * 8 + g * 4
                nc.sync.dma_start(out=rscr2[r0 : r0 + 4, :], in_=rsb)

            def emit_norm(qb, g):
                # normalization muls for the 2 (h) pairs of (qb,g) on the
                # otherwise-idle GpSimd (all operands SBUF)
                for h in range(2):
                    stg = stg_tiles.pop((qb, g, h))
                    rcb = rcbp.tile([128, 512], BF16, tag="rcb",
                                    name=f"rcb_{qb}_{g}_{h}")
                    r0 = qb * 8 + g * 4 + h * 2
                    nc.sync.dma_start(
                        out=rcb[0:32, :],
                        in_=rscr2[r0 : r0 + 1, :].to_broadcast((32, 512)),
                    )
                    nc.sync.dma_start(
                        out=rcb[64:96, :],
                        in_=rscr2[r0 + 1 : r0 + 2, :].to_broadcast((32, 512)),
                    )
                    dst = ctxT_sb[2 * g + h]
                    nc.gpsimd.tensor_mul(
                        out=dst[0:32, qb * 512 : qb * 512 + 512],
                        in0=stg[0:32, :], in1=rcb[0:32, :],
                    )
                    nc.gpsimd.tensor_mul(
                        out=dst[64:96, qb * 512 : qb * 512 + 512],
                        in0=stg[64:96, :], in1=rcb[64:96, :],
                    )

            step = 0
            groups = [(qb, g) for qb in range(4) for g in range(2)]
            for gi, (qb, g) in enumerate(groups):
                ctx_t = [
                    cxp.tile([128, 512], F32, tag="cx",
                             name=f"cx_{qb}_{g}_{h}")
                    for h in range(2)
                ]
                eTs_by_kt = {}
                for kt in range(16):
                    # deferred normalization work for the previous group
                    # (emitted here so its DMA roundtrip latency never
                    # blocks the in-order engine queues)
                    if gi > 0:
                        if kt == 3:
                            emit_recip(*groups[gi - 1])
                        if kt == 9:
                            emit_norm(*groups[gi - 1])
                    # scores: all 4 row-tiled matmuls of this kt adjacent
                    # so they overlap 4-way on the PE
                    scs = []
                    for h in range(2):
                        sc = scp.tile([128, 1024], F32, tag="sc",
                                      name=f"sc_{qb}_{g}_{kt}_{h}")
                        for jj, j in enumerate((2 * h, 2 * h + 1)):
                            nc.tensor.matmul(
                                out=sc[:, jj * 512 : jj * 512 + 512],
                                lhsT=kT_sb[32 * j : 32 * j + 32,
                                           g * S + kt * 128 : g * S + kt * 128 + 128],
                                rhs=qT_sb[32 * j : 32 * j + 32,
                                          g * S + qb * 512 : g * S + qb * 512 + 512],
                                start=True, stop=True,
                                tile_position=(32 * j, 0),
                            )
                        scs.append(sc)
                    eTs = []
                    for h in range(2):
                        eT = etp.tile([128, 1024], BF16, tag="eT")
                        if ACT_PAT[step % 32]:
                            nc.scalar.activation(
                                out=eT, in_=scs[h], func=EXP, scale=SCALE,
                            )
                        else:
                            nc.vector.tensor_scalar(
                                out=eT.bitcast(I16), in0=scs[h],
                                scalar1=EXP_A, scalar2=EXP_B,
                                op0=mybir.AluOpType.mult,
                                op1=mybir.AluOpType.add,
                            )
                        eTs.append(eT)
                        step += 1
                    eTs_by_kt[kt] = eTs
                    if kt >= 1:
                        emit_ctx(qb, g, kt - 1, ctx_t, eTs_by_kt.pop(kt - 1))
                    # pipeline the remaining projections into (qb0, g0):
                    # v st-chunks one kt ahead of their first ctx use, the
                    # late q/k strips (heads 4-7) before g=1 needs them
                    if gi == 0:
                        if deferred_proj and kt >= 1:
                            kind, st = deferred_proj[0]
                            if st <= kt + 1:
                                deferred_proj.pop(0)
                                emit_v_chunk(st)
                        if late_qk and 4 <= kt <= 11:
                            t, nb = late_qk.pop(0)
                            emit_qk_chunk(t, nb)
                emit_ctx(qb, g, 15, ctx_t, eTs_by_kt.pop(15))
                # evict ctx PSUM (bf16 staging, one [97,512] copy per
                # pair); rowsum rows (32/96, in bf16) are gathered via
                # tiny SBUF->SBUF DMAs into a packed [64,32] tile for
                # the reciprocal (row j of 512 -> 16 partitions x 32)
                rsgb = ev.tile([64, 32], BF16, tag="rsgb",
                               name=f"rsgb_{qb}_{g}")
                for h in range(2):
                    stg = stgp.tile([128, 512], BF16, tag="stg",
                                    name=f"stg_{qb}_{g}_{h}")
                    nc.vector.tensor_copy(
                        out=stg[0:97, :], in_=ctx_t[h][0:97, :]
                    )
                    stg_tiles[(qb, g, h)] = stg
                    p0 = h * 32
                    nc.sync.dma_start(out=rsgb[p0 : p0 + 16, :],
                                      in_=stg[32:33, :])
                    nc.sync.dma_start(out=rsgb[p0 + 16 : p0 + 32, :],
                                      in_=stg[96:97, :])
                rsg_tiles[(qb, g)] = rsgb
            # tail normalization for the last group
            emit_recip(3, 1)
            emit_norm(3, 1)

            # ---- out = ctxT^T @ w_out_perm + b_out, via the same rotation
            #      pool (no pool-open barrier); evictions alternate ACT/DVE
            for st in range(16):
                ps = scp.tile([128, 1024], F32, tag="sc", name=f"op_{st}")
                for kk in range(4):
                    nc.tensor.matmul(
                        out=ps[:, 0:256],
                        lhsT=ctxT_sb[kk][:, st * 128 : st * 128 + 128],
                        rhs=wo_sb[:, kk * 256 : kk * 256 + 256],
                        start=(kk == 0), stop=(kk == 3),
                    )
                ot = ev.tile([128, 256], F32, tag="ot")
                if st % 2 == 0:
                    nc.scalar.copy(out=ot, in_=ps[:, 0:256])
                else:
                    nc.vector.tensor_copy(out=ot, in_=ps[:, 0:256])
                nc.sync.dma_start(
                    out=out[st * 128 : st * 128 + 128, :], in_=ot
                )
    if legalize:
        _legalize_sync_waits(nc)
    return nc


_NC_CACHE = None


def _get_nc():
    global _NC_CACHE
    if _NC_CACHE is None:
        _NC_CACHE = _build_nc()
    return _NC_CACHE


def _ks_layout(a, nk, cols):
    """[nk*128, cols] -> [128, nk*cols] with [p, k*cols+c] = a[k*128+p, c]."""
    return np.ascontiguousarray(
        a.reshape(nk, 128, cols).transpose(1, 0, 2).reshape(128, nk * cols)
    )


def _prep_in_maps(x, w_qkv, b_qkv, w_out, b_out):
    x = np.asarray(x, dtype=np.float32)
    w_qkv = np.asarray(w_qkv, dtype=np.float32)
    b_qkv = np.asarray(b_qkv, dtype=np.float32)
    w_out = np.asarray(w_out, dtype=np.float32)
    b_out = np.asarray(b_out, dtype=np.float32)

    # shared (per-core identical) weight layouts
    wqk_l = _ks_layout(w_qkv[:, : 2 * H], 2, 512).astype(NPBF16)

    # v weights: 64-wide slot per head: [v_h (32) | ones-col | 31 zero]
    # (the ones column itself is planted via the bias matmul; v bias is the
    # spec's b_qkv v-slice)
    wpad = np.zeros((H, 264), np.float32)
    bvr = np.zeros((1, 264), np.float32)
    for h in range(NH):
        c0 = h * 33
        wpad[:, c0 : c0 + 32] = w_qkv[:, 2 * H + h * HD : 2 * H + (h + 1) * HD]
        bvr[0, c0 : c0 + 32] = b_qkv[2 * H + h * HD : 2 * H + (h + 1) * HD]
        bvr[0, c0 + 32] = 1.0  # ones column -> rowsum row
    wv_l = _ks_layout(wpad, 2, 264).astype(NPBF16)

    # w_out rows permuted into the ctxT slot layout (zeros in pad slots)
    wo_perm = np.zeros((512, H), np.float32)
    for pair in range(4):
        for side in range(2):
            h = 2 * pair + side
            r0 = pair * 128 + side * 64
            wo_perm[r0 : r0 + 32, :] = w_out[h * HD : (h + 1) * HD, :]
    wo_perm[32, :] = b_out  # multiplied by the ctxT[0] ones row
    wo_l = _ks_layout(wo_perm, 4, 256).astype(NPBF16)

    shared = {
        "wqk": wqk_l,
        "wv": wv_l,
        "bv": bvr.astype(NPBF16),
        "wo": wo_l,
        "bqkc": np.ascontiguousarray(
            b_qkv[: 2 * H].astype(np.float32).reshape(4, 128).T
        ),
        "ones": np.ones((1, 512), NPBF16),
        "zrow": np.concatenate([np.zeros((1, 2048), NPBF16), np.ones((1, 2048), NPBF16)]),
    }
    in_maps = []
    for b in range(B):
        xt = _ks_layout(np.ascontiguousarray(x[b].T), 2, S).astype(NPBF16)
        in_maps.append({"xt": xt, **shared})
    return in_maps


def kernel(x, w_qkv, b_qkv, w_out, b_out):
    in_maps = _prep_in_maps(x, w_qkv, b_qkv, w_out, b_out)
    nc = _get_nc()
    res = run_bass_kernel_spmd(nc, in_maps, list(range(N_CORES)), **TRACE_OPTS)
    global LAST_RESULT
    LAST_RESULT = res
    return np.stack([res.results[b]["out"] for b in range(B)], axis=0)


# revision 15
# speedup vs baseline: 1.1430x; 1.1430x over previous
"""Multi-head self-attention (B=8, S=2048, H=256, NH=8, HD=32) on 8 TRN2 cores.

Strategy: data-parallel over batch — each core computes full MHA for one
batch element; no collectives.

Per-core dataflow (matmuls bf16 in / fp32 PSUM accum):
  - host ships x^T (features on partitions) so no on-device transpose
  - qkT:  q^T,k^T [feat, s] = w_qkv^T @ x — feature-major so each head's
    32 q/k features land on one 32-partition strip
  - attention runs qb(4) x g(2 head-groups) x kt(16) x half(2):
    per kt all 4 row-tiled scores matmuls (tile_position=(32j,0), K=32)
    adjacent so they overlap 4-way, into a 3-deep rotation of [128,1024]
    PSUM tiles, then one exp per half:
      - ACT steps: scalar ACTIVATE Exp (scale folded in)
      - DVE steps: Schraudolph bf16 exp — tensor_scalar mult+add to an
        int16 view of the bf16 eT tile (i16 = rne(s*A + B) IS the bf16
        bit pattern of ~exp(s*scale)); rowsum-normalization cancels the
        systematic part of the approx error per head
    Splitting exp across both engines breaks the single-engine ACT
    bottleneck; the PE runs at the 1.2GHz mid p-state throughout (the
    2.4GHz gate needs >3us gap-free matmul streaks this dataflow can't
    sustain), so scores/ctx cost ~427ns per 512-col matmul stream
  - ctx^T accumulated over kt with 2x column-tiled PE (tile_position
    (0,0)/(0,64)); stationary v blocks carry a ones column so each
    64-wide head slot yields [ctx_h(32) | rowsum(1)]; ctx for kt-1 is
    emitted as a 4-matmul col-mode burst after kt's scores
  - g-outer keeps only 2 ctx accumulator banks live -> scores get a
    3-tile rotation (6 banks) which decouples the scores->exp WAR chain
  - per (qb,g): ctx PSUM evicted to bf16 staging in one [97,512] copy;
    rowsum rows gathered by tiny SBUF->SBUF DMAs into a packed [64,32]
    tile (reciprocal free-size 32), broadcast back via DRAM; the
    normalization multiplies run on the otherwise-idle GpSimd; all
    chain pieces are emitted deferred (during the next group) so DMA
    latency never stalls the in-order engine queues
  - out = ctxT^T @ w_out_perm + b_out as a tail phase; evictions
    alternate ACT/DVE
"""
import numpy as np
import ml_dtypes

import bass_rust
import concourse.bass as bass
import concourse.mybir as mybir
import concourse.tile as tile
from concourse.bass_utils import run_bass_kernel_spmd

BF16 = mybir.dt.bfloat16
F32 = mybir.dt.float32
I16 = mybir.dt.int16
NPBF16 = ml_dtypes.bfloat16

B, S, H = 8, 2048, 256
NH, HD = 8, 32
SCALE = 1.0 / float(np.sqrt(HD))
N_CORES = 8

# Schraudolph bf16 exp constants: i16 = rne(s*A + B) viewed as bf16
# approximates exp(s*SCALE).  A = SCALE * 2^7 / ln2; B = 127*2^7 - c with
# c=1.5 calibrated for min global error (rne rounding confirmed on HW).
EXP_A = SCALE * 128.0 / float(np.log(2.0))
EXP_B = 16256.0 - 1.5

# ACT/DVE exp split: pattern over 32 steps, True -> ACT. 18/32 on ACT.
N_ACT_OF_32 = 18
ACT_PAT = [((i + 1) * N_ACT_OF_32) // 32 - (i * N_ACT_OF_32) // 32 == 1
           for i in range(32)]

# Set by a test harness to collect HW timing: {"trace": bool, "trace_cores": [...]}
TRACE_OPTS = {}
LAST_RESULT = None


def _legalize_sync_waits(nc):
    """The walrus build here rejects >1 sync wait per instruction, but Tile
    freely emits 2-3 (and the exit drain up to ~27).  Move excess waits onto
    same-engine NoOp carriers inserted immediately before the offending
    instruction — identical semantics (the engine blocks on each wait in
    program order)."""
    n = 0
    for f in nc.m.functions:
        for bb in f.blocks:
            insts = bb.instructions  # live list
            i = 0
            while i < len(insts):
                inst = insts[i]
                si = inst.sync_info
                if si is not None and len(si.on_wait) > 1:
                    waits = list(si.on_wait)
                    carriers = []
                    for w in waits[:-1]:
                        carriers.append(
                            mybir.InstNoOp(
                                name=f"{inst.name}-w{n}",
                                sync_info=mybir.SyncInfo(on_wait=[w], on_update=[]),
                                bass_nofuse=True,
                                engine=inst.engine,
                            )
                        )
                        n += 1
                    inst.sync_info = bass_rust.SyncInfo(
                        on_wait=waits[-1:], on_update=list(si.on_update)
                    )
                    insts[i:i] = carriers
                    i += len(carriers)
                i += 1
    return n


def _build_nc(legalize=True):
    nc = bass.Bass()
    xt = nc.dram_tensor("xt", [128, 2 * S], BF16, kind="ExternalInput")
    wqk = nc.dram_tensor("wqk", [128, 2 * 512], BF16, kind="ExternalInput")
    bv = nc.dram_tensor("bv", [1, 264], BF16, kind="ExternalInput")
    wv = nc.dram_tensor("wv", [128, 2 * 264], BF16, kind="ExternalInput")
    wo = nc.dram_tensor("wo", [128, 4 * 256], BF16, kind="ExternalInput")
    bqkc = nc.dram_tensor("bqkc", [128, 4], F32, kind="ExternalInput")
    ones = nc.dram_tensor("ones", [1, 512], BF16, kind="ExternalInput")
    zrow = nc.dram_tensor("zrow", [2, 2048], BF16, kind="ExternalInput")
    out = nc.dram_tensor("out", [S, H], F32, kind="ExternalOutput")
    # scratch for the rowsum-reciprocal broadcast (SBUF APs cannot have a
    # zero partition step, DRAM APs can): one row per (qb, g, h, side)
    rscr2 = nc.dram_tensor("rscr2", [32, 512], BF16)

    EXP = mybir.ActivationFunctionType.Exp

    with tile.TileContext(nc) as tc:
        with (
            tc.tile_pool(name="const", bufs=1) as const,
            tc.tile_pool(name="ev", bufs=6) as ev,
            tc.tile_pool(name="stgp", bufs=4) as stgp,
            tc.tile_pool(name="rcbp", bufs=4) as rcbp,
            tc.tile_pool(name="etp", bufs=8) as etp,
        ):
            # critical-path inputs first: wqk + xt gate the first projections
            wqk_sb = const.tile([128, 2 * 512], BF16, tag="wqk")
            nc.sync.dma_start(out=wqk_sb, in_=wqk[:, :])
            xt_sb = const.tile([128, 2 * S], BF16, tag="xt")
            nc.sync.dma_start(out=xt_sb, in_=xt[:, :])
            wv_sb = const.tile([128, 2 * 264], BF16, tag="wv")
            nc.sync.dma_start(out=wv_sb, in_=wv[:, :])
            bv_sb = const.tile([1, 264], BF16, tag="bv")
            nc.sync.dma_start(out=bv_sb, in_=bv[:, :])
            ones1_sb = const.tile([1, 128], BF16, tag="ones1")
            nc.sync.dma_start(out=ones1_sb, in_=ones[0:1, 0:128])
            bqkc_sb = const.tile([128, 4], F32, tag="bqkc")
            nc.sync.dma_start(out=bqkc_sb, in_=bqkc[:, :])
            wo_sb = const.tile([128, 4 * 256], BF16, tag="wo")
            nc.sync.dma_start(out=wo_sb, in_=wo[:, :])

            qT_sb = const.tile([128, 2 * S], BF16, tag="qT")
            kT_sb = const.tile([128, 2 * S], BF16, tag="kT")
            v_sb = const.tile([128, 16 * 264], BF16, tag="v")
            ctxT_sb = [
                const.tile([128, S], BF16, tag=f"ctxT{k}", name=f"ctxT{k}")
                for k in range(4)
            ]
            # rows 32:64 / 96:128 of each ctxT tile are never written by the
            # normalization muls but are contracted by the output matmul
            # (against zeroed w_out rows) — clear them via broadcast DMA so
            # stale NaN patterns can't poison the accumulation
            for k in range(4):
                if k == 0:
                    # row 32 of tile 0 is all-ones: paired with w_out_perm
                    # row 32 = b_out it adds the output bias for free
                    nc.sync.dma_start(out=ctxT_sb[0][32:33, :], in_=zrow[1:2, :])
                    nc.sync.dma_start(
                        out=ctxT_sb[0][33:64, :],
                        in_=zrow[0:1, :].to_broadcast((31, S)),
                    )
                else:
                    nc.sync.dma_start(
                        out=ctxT_sb[k][32:64, :],
                        in_=zrow[0:1, :].to_broadcast((32, S)),
                    )
                nc.sync.dma_start(
                    out=ctxT_sb[k][96:128, :],
                    in_=zrow[0:1, :].to_broadcast((32, S)),
                )

            # ---- phase 0: HAM warmup — dep-free back-to-back matmuls so the
            # PE clock gate opens before the real work ----
            with tc.tile_pool(name="pp", bufs=4, space="PSUM") as pp:
                warm_sb = const.tile([128, 512], BF16, tag="warm")
                nc.vector.memset(warm_sb, 0.0)
                warm_ps = pp.tile([128, 512], F32, tag="pp")
                for _ in range(6):
                    nc.tensor.matmul(
                        out=warm_ps, lhsT=warm_sb[:, 0:128], rhs=warm_sb[:, :],
                        start=True, stop=True,
                    )

                # ---- phase 1: qT/kT [feature, s] = w_qkv^T @ x; bias folded
                #      into the eviction (per-partition, features-major) ----
                for t in range(4):  # feature tiles: q0,q1,k0,k1
                    for nb in range(4):  # s blocks of 512
                        ps = pp.tile([128, 512], F32, tag="pp")
                        for ks in range(2):
                            nc.tensor.matmul(
                                out=ps,
                                lhsT=wqk_sb[:, ks * 512 + t * 128 : ks * 512 + t * 128 + 128],
                                rhs=xt_sb[:, ks * S + nb * 512 : ks * S + nb * 512 + 512],
                                start=(ks == 0), stop=(ks == 1),
                            )
                        dst = (qT_sb if t < 2 else kT_sb)[
                            :, (t % 2) * S + nb * 512 : (t % 2) * S + nb * 512 + 512
                        ]
                        nc.vector.tensor_scalar_add(
                            out=dst, in0=ps, scalar1=bqkc_sb[:, t : t + 1]
                        )

                # ---- phase 2: v (natural layout, padded 64-wide head slots,
                #      ones column at j=32 for rowsums) ----
                for st in range(16):
                    ps = pp.tile([128, 264], F32, tag="ppv")
                    for ks in range(2):
                        nc.tensor.matmul(
                            out=ps,
                            lhsT=xt_sb[:, ks * S + st * 128 : ks * S + st * 128 + 128],
                            rhs=wv_sb[:, ks * 264 : ks * 264 + 264],
                            start=(ks == 0), stop=False,
                        )
                    # bias row also plants the rowsum ones-columns
                    nc.tensor.matmul(
                        out=ps,
                        lhsT=ones1_sb[0:1, 0:128],
                        rhs=bv_sb[0:1, 0:264],
                        start=False, stop=True,
                    )
                    dst = v_sb[:, st * 264 : st * 264 + 264]
                    nc.scalar.copy(out=dst, in_=ps)

            # ---- phase 3: attention ----
            stg_tiles = {}   # (qb, g, h) -> stg tile
            rsg_tiles = {}   # (qb, g) -> packed bf16 rowsum gather tile

            with (
                tc.tile_pool(name="scp", bufs=3, space="PSUM") as scp,
                tc.tile_pool(name="cxp", bufs=2, space="PSUM") as cxp,
            ):
                def emit_ctx(qb, g, kt, ctx_t, eTs):
                    # ctx accumulation for (g, kt): 4 col-mode matmuls
                    for h in range(2):
                        cps = ctx_t[h]
                        eT = eTs[h]
                        vc = kt * 264 + (4 * g + 2 * h) * 33
                        nc.tensor.matmul(
                            out=cps[0:33, :],
                            lhsT=v_sb[:, vc : vc + 33],
                            rhs=eT[:, 0:512],
                            start=(kt == 0), stop=(kt == 15),
                            tile_position=(0, 0), skip_group_check=True,
                        )
                        nc.tensor.matmul(
                            out=cps[64:97, :],
                            lhsT=v_sb[:, vc + 33 : vc + 66],
                            rhs=eT[:, 512:1024],
                            start=(kt == 0), stop=(kt == 15),
                            tile_position=(0, 64), skip_group_check=True,
                        )

                def emit_recip(qb, g):
                    # rowsum reciprocal for the 4 (h,side) rows of (qb,g),
                    # packed [64,32] (free size 32) so the DVE cost is tiny;
                    # result lands in rscr2 rows for the broadcast DMAs
                    rsgb = rsg_tiles.pop((qb, g))
                    rsgf = ev.tile([64, 32], F32, tag="rsgf")
                    nc.vector.tensor_copy(out=rsgf, in_=rsgb)
                    rsr = ev.tile([64, 32], F32, tag="rsr")
                    nc.vector.reciprocal(out=rsr, in_=rsgf)
                    rsb = ev.tile([64, 32], BF16, tag="rsb")
                    nc.vector.tensor_copy(out=rsb, in_=rsr)
                    r0 = qb * 8 + g * 4
                    nc.sync.dma_start(out=rscr2[r0 : r0 + 4, :], in_=rsb)

                def emit_norm(qb, g, tail=False):
                    # normalization muls for the 2 (h) pairs of (qb,g); on
                    # the otherwise-idle GpSimd mid-stream, on the (faster)
                    # DVE for the tail group where nothing else competes
                    for h in range(2):
                        stg = stg_tiles.pop((qb, g, h))
                        rcb = rcbp.tile([128, 512], BF16, tag="rcb",
                                        name=f"rcb_{qb}_{g}_{h}")
                        r0 = qb * 8 + g * 4 + h * 2
                        nc.sync.dma_start(
                            out=rcb[0:32, :],
                            in_=rscr2[r0 : r0 + 1, :].to_broadcast((32, 512)),
                        )
                        nc.sync.dma_start(
                            out=rcb[64:96, :],
                            in_=rscr2[r0 + 1 : r0 + 2, :].to_broadcast((32, 512)),
                        )
                        eng = nc.vector if tail else nc.gpsimd
                        dst = ctxT_sb[2 * g + h]
                        eng.tensor_mul(
                            out=dst[0:32, qb * 512 : qb * 512 + 512],
                            in0=stg[0:32, :], in1=rcb[0:32, :],
                        )
                        eng.tensor_mul(
                            out=dst[64:96, qb * 512 : qb * 512 + 512],
                            in0=stg[64:96, :], in1=rcb[64:96, :],
                        )

                step = 0
                groups = [(qb, g) for qb in range(4) for g in range(2)]
                for gi, (qb, g) in enumerate(groups):
                    ctx_t = [
                        cxp.tile([128, 512], F32, tag="cx",
                                 name=f"cx_{qb}_{g}_{h}")
                        for h in range(2)
                    ]
                    eTs_by_kt = {}
                    for kt in range(16):
                        # deferred normalization work for the previous group
                        # (emitted here so its DMA roundtrip latency never
                        # blocks the in-order engine queues)
                        if gi > 0:
                            if kt == 3:
                                emit_recip(*groups[gi - 1])
                            if kt == 9:
                                emit_norm(*groups[gi - 1])
                        # scores: all 4 row-tiled matmuls of this kt adjacent
                        # so they overlap 4-way on the PE
                        scs = []
                        for h in range(2):
                            sc = scp.tile([128, 1024], F32, tag="sc",
                                          name=f"sc_{qb}_{g}_{kt}_{h}")
                            for jj, j in enumerate((2 * h, 2 * h + 1)):
                                nc.tensor.matmul(
                                    out=sc[:, jj * 512 : jj * 512 + 512],
                                    lhsT=kT_sb[32 * j : 32 * j + 32,
                                               g * S + kt * 128 : g * S + kt * 128 + 128],
                                    rhs=qT_sb[32 * j : 32 * j + 32,
                                              g * S + qb * 512 : g * S + qb * 512 + 512],
                                    start=True, stop=True,
                                    tile_position=(32 * j, 0),
                                )
                            scs.append(sc)
                        eTs = []
                        for h in range(2):
                            eT = etp.tile([128, 1024], BF16, tag="eT")
                            if ACT_PAT[step % 32]:
                                nc.scalar.activation(
                                    out=eT, in_=scs[h], func=EXP, scale=SCALE,
                                )
                            else:
                                nc.vector.tensor_scalar(
                                    out=eT.bitcast(I16), in0=scs[h],
                                    scalar1=EXP_A, scalar2=EXP_B,
                                    op0=mybir.AluOpType.mult,
                                    op1=mybir.AluOpType.add,
                                )
                            eTs.append(eT)
                            step += 1
                        eTs_by_kt[kt] = eTs
                        if kt >= 1:
                            emit_ctx(qb, g, kt - 1, ctx_t,
                                     eTs_by_kt.pop(kt - 1))
                    emit_ctx(qb, g, 15, ctx_t, eTs_by_kt.pop(15))
                    # evict ctx PSUM (bf16 staging, one [97,512] copy per
                    # pair); rowsum rows (32/96, in bf16) are gathered via
                    # tiny SBUF->SBUF DMAs into a packed [64,32] tile for
                    # the reciprocal (row j of 512 -> 16 partitions x 32)
                    rsgb = ev.tile([64, 32], BF16, tag="rsgb",
                                   name=f"rsgb_{qb}_{g}")
                    for h in range(2):
                        stg = stgp.tile([128, 512], BF16, tag="stg",
                                        name=f"stg_{qb}_{g}_{h}")
                        nc.vector.tensor_copy(
                            out=stg[0:97, :], in_=ctx_t[h][0:97, :]
                        )
                        stg_tiles[(qb, g, h)] = stg
                        p0 = h * 32
                        nc.sync.dma_start(out=rsgb[p0 : p0 + 16, :],
                                          in_=stg[32:33, :])
                        nc.sync.dma_start(out=rsgb[p0 + 16 : p0 + 32, :],
                                          in_=stg[96:97, :])
                    rsg_tiles[(qb, g)] = rsgb
                # tail normalization for the last group
                emit_recip(3, 1)
                emit_norm(3, 1, tail=True)

            # ---- phase 4: out = ctxT^T @ w_out_perm + b_out; evictions
            #      alternate ACT/DVE ----
            with tc.tile_pool(name="op", bufs=4, space="PSUM") as op:
                for st in range(16):
                    ps = op.tile([128, 256], F32, tag="op")
                    for kk in range(4):
                        nc.tensor.matmul(
                            out=ps,
                            lhsT=ctxT_sb[kk][:, st * 128 : st * 128 + 128],
                            rhs=wo_sb[:, kk * 256 : kk * 256 + 256],
                            start=(kk == 0), stop=(kk == 3),
                        )
                    ot = ev.tile([128, 256], F32, tag="ot")
                    if st % 2 == 0:
                        nc.scalar.copy(out=ot, in_=ps)
                    else:
                        nc.vector.tensor_copy(out=ot, in_=ps)
                    nc.sync.dma_start(
                        out=out[st * 128 : st * 128 + 128, :], in_=ot
                    )
    if legalize:
        _legalize_sync_waits(nc)
    return nc


_NC_CACHE = None


def _get_nc():
    global _NC_CACHE
    if _NC_CACHE is None:
        _NC_CACHE = _build_nc()
    return _NC_CACHE


def _ks_layout(a, nk, cols):
    """[nk*128, cols] -> [128, nk*cols] with [p, k*cols+c] = a[k*128+p, c]."""
    return np.ascontiguousarray(
        a.reshape(nk, 128, cols).transpose(1, 0, 2).reshape(128, nk * cols)
    )


def _prep_in_maps(x, w_qkv, b_qkv, w_out, b_out):
    x = np.asarray(x, dtype=np.float32)
    w_qkv = np.asarray(w_qkv, dtype=np.float32)
    b_qkv = np.asarray(b_qkv, dtype=np.float32)
    w_out = np.asarray(w_out, dtype=np.float32)
    b_out = np.asarray(b_out, dtype=np.float32)

    # shared (per-core identical) weight layouts
    wqk_l = _ks_layout(w_qkv[:, : 2 * H], 2, 512).astype(NPBF16)

    # v weights: 64-wide slot per head: [v_h (32) | ones-col | 31 zero]
    # (the ones column itself is planted via the bias matmul; v bias is the
    # spec's b_qkv v-slice)
    wpad = np.zeros((H, 264), np.float32)
    bvr = np.zeros((1, 264), np.float32)
    for h in range(NH):
        c0 = h * 33
        wpad[:, c0 : c0 + 32] = w_qkv[:, 2 * H + h * HD : 2 * H + (h + 1) * HD]
        bvr[0, c0 : c0 + 32] = b_qkv[2 * H + h * HD : 2 * H + (h + 1) * HD]
        bvr[0, c0 + 32] = 1.0  # ones column -> rowsum row
    wv_l = _ks_layout(wpad, 2, 264).astype(NPBF16)

    # w_out rows permuted into the ctxT slot layout (zeros in pad slots)
    wo_perm = np.zeros((512, H), np.float32)
    for pair in range(4):
        for side in range(2):
            h = 2 * pair + side
            r0 = pair * 128 + side * 64
            wo_perm[r0 : r0 + 32, :] = w_out[h * HD : (h + 1) * HD, :]
    wo_perm[32, :] = b_out  # multiplied by the ctxT[0] ones row
    wo_l = _ks_layout(wo_perm, 4, 256).astype(NPBF16)

    shared = {
        "wqk": wqk_l,
        "wv": wv_l,
        "bv": bvr.astype(NPBF16),
        "wo": wo_l,
        "bqkc": np.ascontiguousarray(
            b_qkv[: 2 * H].astype(np.float32).reshape(4, 128).T
        ),
        "ones": np.ones((1, 512), NPBF16),
        "zrow": np.concatenate([np.zeros((1, 2048), NPBF16), np.ones((1, 2048), NPBF16)]),
    }
    in_maps = []
    for b in range(B):
        xt = _ks_layout(np.ascontiguousarray(x[b].T), 2, S).astype(NPBF16)
        in_maps.append({"xt": xt, **shared})
    return in_maps


def kernel(x, w_qkv, b_qkv, w_out, b_out):
    in_maps = _prep_in_maps(x, w_qkv, b_qkv, w_out, b_out)
    nc = _get_nc()
    res = run_bass_kernel_spmd(nc, in_maps, list(range(N_CORES)), **TRACE_OPTS)
    global LAST_RESULT
    LAST_RESULT = res
    return np.stack([res.results[b]["out"] for b in range(B)], axis=0)


# revision 18
# speedup vs baseline: 1.1445x; 1.0013x over previous
"""Multi-head self-attention (B=8, S=2048, H=256, NH=8, HD=32) on 8 TRN2 cores.

Strategy: data-parallel over batch — each core computes full MHA for one
batch element; no collectives.

Per-core dataflow (matmuls bf16 in / fp32 PSUM accum):
  - host ships x^T (features on partitions) so no on-device transpose
  - qkT:  q^T,k^T [feat, s] = w_qkv^T @ x — feature-major so each head's
    32 q/k features land on one 32-partition strip
  - attention runs qb(4) x g(2 head-groups) x kt(16) x half(2):
    per kt all 4 row-tiled scores matmuls (tile_position=(32j,0), K=32)
    adjacent so they overlap 4-way, into a 3-deep rotation of [128,1024]
    PSUM tiles, then one exp per half:
      - ACT steps: scalar ACTIVATE Exp (scale folded in)
      - DVE steps: Schraudolph bf16 exp — tensor_scalar mult+add to an
        int16 view of the bf16 eT tile (i16 = rne(s*A + B) IS the bf16
        bit pattern of ~exp(s*scale)); rowsum-normalization cancels the
        systematic part of the approx error per head
    Splitting exp across both engines breaks the single-engine ACT
    bottleneck; the PE runs at the 1.2GHz mid p-state throughout (the
    2.4GHz gate needs >3us gap-free matmul streaks this dataflow can't
    sustain), so scores/ctx cost ~427ns per 512-col matmul stream
  - ctx^T accumulated over kt with 2x column-tiled PE (tile_position
    (0,0)/(0,64)); stationary v blocks carry a ones column so each
    64-wide head slot yields [ctx_h(32) | rowsum(1)]; ctx for kt-1 is
    emitted as a 4-matmul col-mode burst after kt's scores
  - g-outer keeps only 2 ctx accumulator banks live -> scores get a
    3-tile rotation (6 banks) which decouples the scores->exp WAR chain
  - per (qb,g): ctx PSUM evicted to bf16 staging in one [97,512] copy;
    rowsum rows gathered by tiny SBUF->SBUF DMAs into a packed [64,32]
    tile (reciprocal free-size 32), broadcast back via DRAM; the
    normalization multiplies run on the otherwise-idle GpSimd; all
    chain pieces are emitted deferred (during the next group) so DMA
    latency never stalls the in-order engine queues
  - out = ctxT^T @ w_out_perm + b_out as a tail phase; evictions
    alternate ACT/DVE
"""
import numpy as np
import ml_dtypes

import bass_rust
import concourse.bass as bass
import concourse.mybir as mybir
import concourse.tile as tile
from concourse.bass_utils import run_bass_kernel_spmd

BF16 = mybir.dt.bfloat16
F32 = mybir.dt.float32
I16 = mybir.dt.int16
NPBF16 = ml_dtypes.bfloat16

B, S, H = 8, 2048, 256
NH, HD = 8, 32
SCALE = 1.0 / float(np.sqrt(HD))
N_CORES = 8

# Schraudolph bf16 exp constants: i16 = rne(s*A + B) viewed as bf16
# approximates exp(s*SCALE).  A = SCALE * 2^7 / ln2; B = 127*2^7 - c with
# c=1.5 calibrated for min global error (rne rounding confirmed on HW).
EXP_A = SCALE * 128.0 / float(np.log(2.0))
EXP_B = 16256.0 - 1.5

# ACT/DVE exp split: pattern over 32 steps, True -> ACT. 18/32 on ACT.
N_ACT_OF_32 = 18
ACT_PAT = [((i + 1) * N_ACT_OF_32) // 32 - (i * N_ACT_OF_32) // 32 == 1
           for i in range(32)]

# Set by a test harness to collect HW timing: {"trace": bool, "trace_cores": [...]}
TRACE_OPTS = {}
LAST_RESULT = None


def _legalize_sync_waits(nc):
    """The walrus build here rejects >1 sync wait per instruction, but Tile
    freely emits 2-3 (and the exit drain up to ~27).  Move excess waits onto
    same-engine NoOp carriers inserted immediately before the offending
    instruction — identical semantics (the engine blocks on each wait in
    program order)."""
    n = 0
    for f in nc.m.functions:
        for bb in f.blocks:
            insts = bb.instructions  # live list
            i = 0
            while i < len(insts):
                inst = insts[i]
                si = inst.sync_info
                if si is not None and len(si.on_wait) > 1:
                    waits = list(si.on_wait)
                    carriers = []
                    for w in waits[:-1]:
                        carriers.append(
                            mybir.InstNoOp(
                                name=f"{inst.name}-w{n}",
                                sync_info=mybir.SyncInfo(on_wait=[w], on_update=[]),
                                bass_nofuse=True,
                                engine=inst.engine,
                            )
                        )
                        n += 1
                    inst.sync_info = bass_rust.SyncInfo(
                        on_wait=waits[-1:], on_update=list(si.on_update)
                    )
                    insts[i:i] = carriers
                    i += len(carriers)
                i += 1
    return n


def _build_nc(legalize=True):
    nc = bass.Bass()
    xt = nc.dram_tensor("xt", [128, 2 * S], BF16, kind="ExternalInput")
    wqk = nc.dram_tensor("wqk", [128, 2 * 512], BF16, kind="ExternalInput")
    bv = nc.dram_tensor("bv", [1, 264], BF16, kind="ExternalInput")
    wv = nc.dram_tensor("wv", [128, 2 * 264], BF16, kind="ExternalInput")
    wo = nc.dram_tensor("wo", [128, 4 * 256], BF16, kind="ExternalInput")
    bqkc = nc.dram_tensor("bqkc", [128, 4], F32, kind="ExternalInput")
    ones = nc.dram_tensor("ones", [1, 512], BF16, kind="ExternalInput")
    zrow = nc.dram_tensor("zrow", [2, 2048], BF16, kind="ExternalInput")
    out = nc.dram_tensor("out", [S, H], F32, kind="ExternalOutput")
    # scratch for the rowsum-reciprocal broadcast (SBUF APs cannot have a
    # zero partition step, DRAM APs can): one row per (qb, g, h, side)
    rscr2 = nc.dram_tensor("rscr2", [32, 512], BF16)

    EXP = mybir.ActivationFunctionType.Exp

    with tile.TileContext(nc) as tc:
        with (
            tc.tile_pool(name="const", bufs=1) as const,
            tc.tile_pool(name="ev", bufs=6) as ev,
            tc.tile_pool(name="stgp", bufs=4) as stgp,
            tc.tile_pool(name="rcbp", bufs=4) as rcbp,
            tc.tile_pool(name="etp", bufs=8) as etp,
        ):
            # critical-path inputs first: wqk + xt gate the first projections
            wqk_sb = const.tile([128, 2 * 512], BF16, tag="wqk")
            nc.sync.dma_start(out=wqk_sb, in_=wqk[:, :])
            # xt split so phase-1 nb=0 can start after the first two pieces
            xt_sb = const.tile([128, 2 * S], BF16, tag="xt")
            nc.sync.dma_start(out=xt_sb[:, 0:512], in_=xt[:, 0:512])
            nc.sync.dma_start(out=xt_sb[:, S : S + 512], in_=xt[:, S : S + 512])
            nc.sync.dma_start(out=xt_sb[:, 512:S], in_=xt[:, 512:S])
            nc.sync.dma_start(out=xt_sb[:, S + 512 : 2 * S], in_=xt[:, S + 512 : 2 * S])
            wv_sb = const.tile([128, 2 * 264], BF16, tag="wv")
            nc.sync.dma_start(out=wv_sb, in_=wv[:, :])
            bv_sb = const.tile([1, 264], BF16, tag="bv")
            nc.sync.dma_start(out=bv_sb, in_=bv[:, :])
            ones1_sb = const.tile([1, 128], BF16, tag="ones1")
            nc.sync.dma_start(out=ones1_sb, in_=ones[0:1, 0:128])
            bqkc_sb = const.tile([128, 4], F32, tag="bqkc")
            nc.sync.dma_start(out=bqkc_sb, in_=bqkc[:, :])
            wo_sb = const.tile([128, 4 * 256], BF16, tag="wo")
            nc.sync.dma_start(out=wo_sb, in_=wo[:, :])

            qT_sb = const.tile([128, 2 * S], BF16, tag="qT")
            kT_sb = const.tile([128, 2 * S], BF16, tag="kT")
            v_sb = const.tile([128, 16 * 264], BF16, tag="v")
            ctxT_sb = [
                const.tile([128, S], BF16, tag=f"ctxT{k}", name=f"ctxT{k}")
                for k in range(4)
            ]
            # rows 32:64 / 96:128 of each ctxT tile are never written by the
            # normalization muls but are contracted by the output matmul
            # (against zeroed w_out rows) — clear them via broadcast DMA so
            # stale NaN patterns can't poison the accumulation
            for k in range(4):
                if k == 0:
                    # row 32 of tile 0 is all-ones: paired with w_out_perm
                    # row 32 = b_out it adds the output bias for free
                    nc.sync.dma_start(out=ctxT_sb[0][32:33, :], in_=zrow[1:2, :])
                    nc.sync.dma_start(
                        out=ctxT_sb[0][33:64, :],
                        in_=zrow[0:1, :].to_broadcast((31, S)),
                    )
                else:
                    nc.sync.dma_start(
                        out=ctxT_sb[k][32:64, :],
                        in_=zrow[0:1, :].to_broadcast((32, S)),
                    )
                nc.sync.dma_start(
                    out=ctxT_sb[k][96:128, :],
                    in_=zrow[0:1, :].to_broadcast((32, S)),
                )

            # ---- phase 0: HAM warmup — dep-free back-to-back matmuls so the
            # PE clock gate opens before the real work ----
            with tc.tile_pool(name="pp", bufs=4, space="PSUM") as pp:
                warm_sb = const.tile([128, 512], BF16, tag="warm")
                nc.vector.memset(warm_sb, 0.0)
                warm_ps = pp.tile([128, 512], F32, tag="pp")
                for _ in range(2):
                    nc.tensor.matmul(
                        out=warm_ps, lhsT=warm_sb[:, 0:128], rhs=warm_sb[:, :],
                        start=True, stop=True,
                    )

                # ---- phase 1: qT/kT [feature, s] = w_qkv^T @ x; bias folded
                #      into the eviction (per-partition, features-major) ----
                for t in range(4):  # feature tiles: q0,q1,k0,k1
                    for nb in range(4):  # s blocks of 512
                        ps = pp.tile([128, 512], F32, tag="pp")
                        for ks in range(2):
                            nc.tensor.matmul(
                                out=ps,
                                lhsT=wqk_sb[:, ks * 512 + t * 128 : ks * 512 + t * 128 + 128],
                                rhs=xt_sb[:, ks * S + nb * 512 : ks * S + nb * 512 + 512],
                                start=(ks == 0), stop=(ks == 1),
                            )
                        dst = (qT_sb if t < 2 else kT_sb)[
                            :, (t % 2) * S + nb * 512 : (t % 2) * S + nb * 512 + 512
                        ]
                        nc.vector.tensor_scalar_add(
                            out=dst, in0=ps, scalar1=bqkc_sb[:, t : t + 1]
                        )

                # ---- phase 2: v (natural layout, padded 64-wide head slots,
                #      ones column at j=32 for rowsums) ----
                for st in range(16):
                    ps = pp.tile([128, 264], F32, tag="ppv")
                    for ks in range(2):
                        nc.tensor.matmul(
                            out=ps,
                            lhsT=xt_sb[:, ks * S + st * 128 : ks * S + st * 128 + 128],
                            rhs=wv_sb[:, ks * 264 : ks * 264 + 264],
                            start=(ks == 0), stop=False,
                        )
                    # bias row also plants the rowsum ones-columns
                    nc.tensor.matmul(
                        out=ps,
                        lhsT=ones1_sb[0:1, 0:128],
                        rhs=bv_sb[0:1, 0:264],
                        start=False, stop=True,
                    )
                    dst = v_sb[:, st * 264 : st * 264 + 264]
                    nc.scalar.copy(out=dst, in_=ps)

            # ---- phase 3: attention ----
            stg_tiles = {}   # (qb, g, h) -> stg tile
            rsg_tiles = {}   # (qb, g) -> packed bf16 rowsum gather tile

            with (
                tc.tile_pool(name="scp", bufs=3, space="PSUM") as scp,
                tc.tile_pool(name="cxp", bufs=2, space="PSUM") as cxp,
            ):
                def emit_ctx(qb, g, kt, ctx_t, eTs):
                    # ctx accumulation for (g, kt): 4 col-mode matmuls
                    for h in range(2):
                        cps = ctx_t[h]
                        eT = eTs[h]
                        vc = kt * 264 + (4 * g + 2 * h) * 33
                        nc.tensor.matmul(
                            out=cps[0:33, :],
                            lhsT=v_sb[:, vc : vc + 33],
                            rhs=eT[:, 0:512],
                            start=(kt == 0), stop=(kt == 15),
                            tile_position=(0, 0), skip_group_check=True,
                        )
                        nc.tensor.matmul(
                            out=cps[64:97, :],
                            lhsT=v_sb[:, vc + 33 : vc + 66],
                            rhs=eT[:, 512:1024],
                            start=(kt == 0), stop=(kt == 15),
                            tile_position=(0, 64), skip_group_check=True,
                        )

                def emit_recip(qb, g):
                    # rowsum reciprocal for the 4 (h,side) rows of (qb,g),
                    # packed [64,32] (free size 32) so the DVE cost is tiny;
                    # result lands in rscr2 rows for the broadcast DMAs
                    rsgb = rsg_tiles.pop((qb, g))
                    rsgf = ev.tile([64, 32], F32, tag="rsgf")
                    nc.vector.tensor_copy(out=rsgf, in_=rsgb)
                    rsr = ev.tile([64, 32], F32, tag="rsr")
                    nc.vector.reciprocal(out=rsr, in_=rsgf)
                    rsb = ev.tile([64, 32], BF16, tag="rsb")
                    nc.vector.tensor_copy(out=rsb, in_=rsr)
                    r0 = qb * 8 + g * 4
                    nc.sync.dma_start(out=rscr2[r0 : r0 + 4, :], in_=rsb)

                def emit_norm(qb, g, tail=False):
                    # normalization muls for the 2 (h) pairs of (qb,g); on
                    # the otherwise-idle GpSimd mid-stream, on the (faster)
                    # DVE for the tail group where nothing else competes
                    for h in range(2):
                        stg = stg_tiles.pop((qb, g, h))
                        rcb = rcbp.tile([128, 512], BF16, tag="rcb",
                                        name=f"rcb_{qb}_{g}_{h}")
                        r0 = qb * 8 + g * 4 + h * 2
                        nc.sync.dma_start(
                            out=rcb[0:32, :],
                            in_=rscr2[r0 : r0 + 1, :].to_broadcast((32, 512)),
                        )
                        nc.sync.dma_start(
                            out=rcb[64:96, :],
                            in_=rscr2[r0 + 1 : r0 + 2, :].to_broadcast((32, 512)),
                        )
                        eng = nc.vector if tail else nc.gpsimd
                        dst = ctxT_sb[2 * g + h]
                        eng.tensor_mul(
                            out=dst[0:32, qb * 512 : qb * 512 + 512],
                            in0=stg[0:32, :], in1=rcb[0:32, :],
                        )
                        eng.tensor_mul(
                            out=dst[64:96, qb * 512 : qb * 512 + 512],
                            in0=stg[64:96, :], in1=rcb[64:96, :],
                        )

                step = 0
                groups = [(qb, g) for qb in range(4) for g in range(2)]
                for gi, (qb, g) in enumerate(groups):
                    ctx_t = [
                        cxp.tile([128, 512], F32, tag="cx",
                                 name=f"cx_{qb}_{g}_{h}")
                        for h in range(2)
                    ]
                    eTs_by_kt = {}
                    for kt in range(16):
                        # deferred normalization work for the previous group
                        # (emitted here so its DMA roundtrip latency never
                        # blocks the in-order engine queues)
                        if gi > 0:
                            if kt == 3:
                                emit_recip(*groups[gi - 1])
                            if kt == 9:
                                emit_norm(*groups[gi - 1])
                        # scores: all 4 row-tiled matmuls of this kt adjacent
                        # so they overlap 4-way on the PE
                        scs = []
                        for h in range(2):
                            sc = scp.tile([128, 1024], F32, tag="sc",
                                          name=f"sc_{qb}_{g}_{kt}_{h}")
                            for jj, j in enumerate((2 * h, 2 * h + 1)):
                                nc.tensor.matmul(
                                    out=sc[:, jj * 512 : jj * 512 + 512],
                                    lhsT=kT_sb[32 * j : 32 * j + 32,
                                               g * S + kt * 128 : g * S + kt * 128 + 128],
                                    rhs=qT_sb[32 * j : 32 * j + 32,
                                              g * S + qb * 512 : g * S + qb * 512 + 512],
                                    start=True, stop=True,
                                    tile_position=(32 * j, 0),
                                )
                            scs.append(sc)
                        eTs = []
                        for h in range(2):
                            eT = etp.tile([128, 1024], BF16, tag="eT")
                            if ACT_PAT[step % 32]:
                                nc.scalar.activation(
                                    out=eT, in_=scs[h], func=EXP, scale=SCALE,
                                )
                            else:
                                nc.vector.tensor_scalar(
                                    out=eT.bitcast(I16), in0=scs[h],
                                    scalar1=EXP_A, scalar2=EXP_B,
                                    op0=mybir.AluOpType.mult,
                                    op1=mybir.AluOpType.add,
                                )
                            eTs.append(eT)
                            step += 1
                        eTs_by_kt[kt] = eTs
                        if kt >= 1:
                            emit_ctx(qb, g, kt - 1, ctx_t,
                                     eTs_by_kt.pop(kt - 1))
                    emit_ctx(qb, g, 15, ctx_t, eTs_by_kt.pop(15))
                    # evict ctx PSUM (bf16 staging, one [97,512] copy per
                    # pair); rowsum rows (32/96, in bf16) are gathered via
                    # tiny SBUF->SBUF DMAs into a packed [64,32] tile for
                    # the reciprocal (row j of 512 -> 16 partitions x 32)
                    rsgb = ev.tile([64, 32], BF16, tag="rsgb",
                                   name=f"rsgb_{qb}_{g}")
                    for h in range(2):
                        stg = stgp.tile([128, 512], BF16, tag="stg",
                                        name=f"stg_{qb}_{g}_{h}")
                        nc.vector.tensor_copy(
                            out=stg[0:97, :], in_=ctx_t[h][0:97, :]
                        )
                        stg_tiles[(qb, g, h)] = stg
                        p0 = h * 32
                        nc.sync.dma_start(out=rsgb[p0 : p0 + 16, :],
                                          in_=stg[32:33, :])
                        nc.sync.dma_start(out=rsgb[p0 + 16 : p0 + 32, :],
                                          in_=stg[96:97, :])
                    rsg_tiles[(qb, g)] = rsgb
                # tail normalization for the last group
                emit_recip(3, 1)
                emit_norm(3, 1, tail=True)

            # ---- phase 4: out = ctxT^T @ w_out_perm + b_out; evictions
            #      alternate ACT/DVE ----
            with tc.tile_pool(name="op", bufs=4, space="PSUM") as op:
                for st in range(16):
                    ps = op.tile([128, 256], F32, tag="op")
                    for kk in range(4):
                        nc.tensor.matmul(
                            out=ps,
                            lhsT=ctxT_sb[kk][:, st * 128 : st * 128 + 128],
                            rhs=wo_sb[:, kk * 256 : kk * 256 + 256],
                            start=(kk == 0), stop=(kk == 3),
                        )
                    # evictions alternate ACT/DVE; the out DMAs are issued
                    # from ACT (HWDGE) and GpSimd (SWDGE) so the tail never
                    # serializes on the sync queue's ~600ns/issue
                    ot = ev.tile([128, 256], F32, tag="ot")
                    if st % 2 == 0:
                        nc.scalar.copy(out=ot, in_=ps)
                        nc.scalar.dma_start(
                            out=out[st * 128 : st * 128 + 128, :], in_=ot
                        )
                    else:
                        nc.vector.tensor_copy(out=ot, in_=ps)
                        nc.gpsimd.dma_start(
                            out=out[st * 128 : st * 128 + 128, :], in_=ot
                        )
    if legalize:
        _legalize_sync_waits(nc)
    return nc


_NC_CACHE = None


def _get_nc():
    global _NC_CACHE
    if _NC_CACHE is None:
        _NC_CACHE = _build_nc()
    return _NC_CACHE


def _ks_layout(a, nk, cols):
    """[nk*128, cols] -> [128, nk*cols] with [p, k*cols+c] = a[k*128+p, c]."""
    return np.ascontiguousarray(
        a.reshape(nk, 128, cols).transpose(1, 0, 2).reshape(128, nk * cols)
    )


def _prep_in_maps(x, w_qkv, b_qkv, w_out, b_out):
    x = np.asarray(x, dtype=np.float32)
    w_qkv = np.asarray(w_qkv, dtype=np.float32)
    b_qkv = np.asarray(b_qkv, dtype=np.float32)
    w_out = np.asarray(w_out, dtype=np.float32)
    b_out = np.asarray(b_out, dtype=np.float32)

    # shared (per-core identical) weight layouts
    wqk_l = _ks_layout(w_qkv[:, : 2 * H], 2, 512).astype(NPBF16)

    # v weights: 64-wide slot per head: [v_h (32) | ones-col | 31 zero]
    # (the ones column itself is planted via the bias matmul; v bias is the
    # spec's b_qkv v-slice)
    wpad = np.zeros((H, 264), np.float32)
    bvr = np.zeros((1, 264), np.float32)
    for h in range(NH):
        c0 = h * 33
        wpad[:, c0 : c0 + 32] = w_qkv[:, 2 * H + h * HD : 2 * H + (h + 1) * HD]
        bvr[0, c0 : c0 + 32] = b_qkv[2 * H + h * HD : 2 * H + (h + 1) * HD]
        bvr[0, c0 + 32] = 1.0  # ones column -> rowsum row
    wv_l = _ks_layout(wpad, 2, 264).astype(NPBF16)

    # w_out rows permuted into the ctxT slot layout (zeros in pad slots)
    wo_perm = np.zeros((512, H), np.float32)
    for pair in range(4):
        for side in range(2):
            h = 2 * pair + side
            r0 = pair * 128 + side * 64
            wo_perm[r0 : r0 + 32, :] = w_out[h * HD : (h + 1) * HD, :]
    wo_perm[32, :] = b_out  # multiplied by the ctxT[0] ones row
    wo_l = _ks_layout(wo_perm, 4, 256).astype(NPBF16)

    shared = {
        "wqk": wqk_l,
        "wv": wv_l,
        "bv": bvr.astype(NPBF16),
        "wo": wo_l,
        "bqkc": np.ascontiguousarray(
            b_qkv[: 2 * H].astype(np.float32).reshape(4, 128).T
        ),
        "ones": np.ones((1, 512), NPBF16),
        "zrow": np.concatenate([np.zeros((1, 2048), NPBF16), np.ones((1, 2048), NPBF16)]),
    }
    in_maps = []
    for b in range(B):
        xt = _ks_layout(np.ascontiguousarray(x[b].T), 2, S).astype(NPBF16)
        in_maps.append({"xt": xt, **shared})
    return in_maps


def kernel(x, w_qkv, b_qkv, w_out, b_out):
    in_maps = _prep_in_maps(x, w_qkv, b_qkv, w_out, b_out)
    nc = _get_nc()
    res = run_bass_kernel_spmd(nc, in_maps, list(range(N_CORES)), **TRACE_OPTS)
    global LAST_RESULT
    LAST_RESULT = res
    return np.stack([res.results[b]["out"] for b in range(B)], axis=0)


# revision 19
# speedup vs baseline: 1.1618x; 1.0151x over previous
"""Multi-head self-attention (B=8, S=2048, H=256, NH=8, HD=32) on 8 TRN2 cores.

Strategy: data-parallel over batch — each core computes full MHA for one
batch element; no collectives.

Per-core dataflow (matmuls bf16 in / fp32 PSUM accum):
  - host ships x^T (features on partitions) so no on-device transpose
  - qkT:  q^T,k^T [feat, s] = w_qkv^T @ x — feature-major so each head's
    32 q/k features land on one 32-partition strip
  - attention runs qb(4) x g(2 head-groups) x kt(16) x half(2):
    per kt all 4 row-tiled scores matmuls (tile_position=(32j,0), K=32)
    adjacent so they overlap 4-way, into a 3-deep rotation of [128,1024]
    PSUM tiles, then one exp per half:
      - ACT steps: scalar ACTIVATE Exp (scale folded in)
      - DVE steps: Schraudolph bf16 exp — tensor_scalar mult+add to an
        int16 view of the bf16 eT tile (i16 = rne(s*A + B) IS the bf16
        bit pattern of ~exp(s*scale)); rowsum-normalization cancels the
        systematic part of the approx error per head
    Splitting exp across both engines breaks the single-engine ACT
    bottleneck; the PE runs at the 1.2GHz mid p-state throughout (the
    2.4GHz gate needs >3us gap-free matmul streaks this dataflow can't
    sustain), so scores/ctx cost ~427ns per 512-col matmul stream
  - ctx^T accumulated over kt with 2x column-tiled PE (tile_position
    (0,0)/(0,64)); stationary v blocks carry a ones column so each
    64-wide head slot yields [ctx_h(32) | rowsum(1)]; ctx for kt-1 is
    emitted as a 4-matmul col-mode burst after kt's scores
  - g-outer keeps only 2 ctx accumulator banks live -> scores get a
    3-tile rotation (6 banks) which decouples the scores->exp WAR chain
  - per (qb,g): ctx PSUM evicted to bf16 staging in one [97,512] copy;
    rowsum rows gathered by tiny SBUF->SBUF DMAs into a packed [64,32]
    tile (reciprocal free-size 32), broadcast back via DRAM; the
    normalization multiplies run on the otherwise-idle GpSimd; all
    chain pieces are emitted deferred (during the next group) so DMA
    latency never stalls the in-order engine queues
  - out = ctxT^T @ w_out_perm + b_out as a tail phase; evictions
    alternate ACT/DVE
"""
import numpy as np
import ml_dtypes

import bass_rust
import concourse.bass as bass
import concourse.mybir as mybir
import concourse.tile as tile
from concourse.bass_utils import run_bass_kernel_spmd

BF16 = mybir.dt.bfloat16
F32 = mybir.dt.float32
I16 = mybir.dt.int16
NPBF16 = ml_dtypes.bfloat16

B, S, H = 8, 2048, 256
NH, HD = 8, 32
SCALE = 1.0 / float(np.sqrt(HD))
N_CORES = 8

# Schraudolph bf16 exp constants: i16 = rne(s*A + B) viewed as bf16
# approximates exp(s*SCALE).  A = SCALE * 2^7 / ln2; B = 127*2^7 - c with
# c=1.5 calibrated for min global error (rne rounding confirmed on HW).
EXP_A = SCALE * 128.0 / float(np.log(2.0))
EXP_B = 16256.0 - 1.5

# ACT/DVE exp split: pattern over 32 steps, True -> ACT. 18/32 on ACT.
N_ACT_OF_32 = 18
ACT_PAT = [((i + 1) * N_ACT_OF_32) // 32 - (i * N_ACT_OF_32) // 32 == 1
           for i in range(32)]

# Set by a test harness to collect HW timing: {"trace": bool, "trace_cores": [...]}
TRACE_OPTS = {}
LAST_RESULT = None


def _legalize_sync_waits(nc):
    """The walrus build here rejects >1 sync wait per instruction, but Tile
    freely emits 2-3 (and the exit drain up to ~27).  Move excess waits onto
    same-engine NoOp carriers inserted immediately before the offending
    instruction — identical semantics (the engine blocks on each wait in
    program order)."""
    n = 0
    for f in nc.m.functions:
        for bb in f.blocks:
            insts = bb.instructions  # live list
            i = 0
            while i < len(insts):
                inst = insts[i]
                si = inst.sync_info
                if si is not None and len(si.on_wait) > 1:
                    waits = list(si.on_wait)
                    carriers = []
                    for w in waits[:-1]:
                        carriers.append(
                            mybir.InstNoOp(
                                name=f"{inst.name}-w{n}",
                                sync_info=mybir.SyncInfo(on_wait=[w], on_update=[]),
                                bass_nofuse=True,
                                engine=inst.engine,
                            )
                        )
                        n += 1
                    inst.sync_info = bass_rust.SyncInfo(
                        on_wait=waits[-1:], on_update=list(si.on_update)
                    )
                    insts[i:i] = carriers
                    i += len(carriers)
                i += 1
    return n


def _build_nc(legalize=True):
    nc = bass.Bass()
    xt = nc.dram_tensor("xt", [128, 2 * S], BF16, kind="ExternalInput")
    wqk = nc.dram_tensor("wqk", [128, 2 * 512], BF16, kind="ExternalInput")
    bv = nc.dram_tensor("bv", [1, 264], BF16, kind="ExternalInput")
    wv = nc.dram_tensor("wv", [128, 2 * 264], BF16, kind="ExternalInput")
    wo = nc.dram_tensor("wo", [128, 4 * 256], BF16, kind="ExternalInput")
    bqkc = nc.dram_tensor("bqkc", [128, 4], F32, kind="ExternalInput")
    ones = nc.dram_tensor("ones", [1, 512], BF16, kind="ExternalInput")
    zrow = nc.dram_tensor("zrow", [2, 2048], BF16, kind="ExternalInput")
    out = nc.dram_tensor("out", [S, H], F32, kind="ExternalOutput")
    # scratch for the rowsum-reciprocal broadcast (SBUF APs cannot have a
    # zero partition step, DRAM APs can): one row per (qb, g, h, side)
    rscr2 = nc.dram_tensor("rscr2", [32, 512], BF16)

    EXP = mybir.ActivationFunctionType.Exp

    with tile.TileContext(nc) as tc:
        with (
            tc.tile_pool(name="const", bufs=1) as const,
            tc.tile_pool(name="ev", bufs=6) as ev,
            tc.tile_pool(name="stgp", bufs=4) as stgp,
            tc.tile_pool(name="rcbp", bufs=4) as rcbp,
            tc.tile_pool(name="etp", bufs=8) as etp,
        ):
            # critical-path inputs first: wqk + xt gate the first projections
            wqk_sb = const.tile([128, 2 * 512], BF16, tag="wqk")
            nc.sync.dma_start(out=wqk_sb, in_=wqk[:, :])
            # xt split so phase-1 nb=0 can start after the first two pieces
            xt_sb = const.tile([128, 2 * S], BF16, tag="xt")
            nc.sync.dma_start(out=xt_sb[:, 0:512], in_=xt[:, 0:512])
            nc.sync.dma_start(out=xt_sb[:, S : S + 512], in_=xt[:, S : S + 512])
            nc.sync.dma_start(out=xt_sb[:, 512:S], in_=xt[:, 512:S])
            nc.sync.dma_start(out=xt_sb[:, S + 512 : 2 * S], in_=xt[:, S + 512 : 2 * S])
            wv_sb = const.tile([128, 2 * 264], BF16, tag="wv")
            nc.sync.dma_start(out=wv_sb, in_=wv[:, :])
            bv_sb = const.tile([1, 264], BF16, tag="bv")
            nc.sync.dma_start(out=bv_sb, in_=bv[:, :])
            ones1_sb = const.tile([1, 128], BF16, tag="ones1")
            nc.sync.dma_start(out=ones1_sb, in_=ones[0:1, 0:128])
            bqkc_sb = const.tile([128, 4], F32, tag="bqkc")
            nc.sync.dma_start(out=bqkc_sb, in_=bqkc[:, :])
            wo_sb = const.tile([128, 4 * 256], BF16, tag="wo")
            nc.sync.dma_start(out=wo_sb, in_=wo[:, :])

            qT_sb = const.tile([128, 2 * S], BF16, tag="qT")
            kT_sb = const.tile([128, 2 * S], BF16, tag="kT")
            v_sb = const.tile([128, 16 * 264], BF16, tag="v")
            ctxT_sb = [
                const.tile([128, S], BF16, tag=f"ctxT{k}", name=f"ctxT{k}")
                for k in range(4)
            ]
            # rows 32:64 / 96:128 of each ctxT tile are never written by the
            # normalization muls but are contracted by the output matmul
            # (against zeroed w_out rows) — clear them via broadcast DMA so
            # stale NaN patterns can't poison the accumulation
            for k in range(4):
                if k == 0:
                    # row 32 of tile 0 is all-ones: paired with w_out_perm
                    # row 32 = b_out it adds the output bias for free
                    nc.sync.dma_start(out=ctxT_sb[0][32:33, :], in_=zrow[1:2, :])
                    nc.sync.dma_start(
                        out=ctxT_sb[0][33:64, :],
                        in_=zrow[0:1, :].to_broadcast((31, S)),
                    )
                else:
                    nc.sync.dma_start(
                        out=ctxT_sb[k][32:64, :],
                        in_=zrow[0:1, :].to_broadcast((32, S)),
                    )
                nc.sync.dma_start(
                    out=ctxT_sb[k][96:128, :],
                    in_=zrow[0:1, :].to_broadcast((32, S)),
                )

            # ---- phase 0: HAM warmup — dep-free back-to-back matmuls so the
            # PE clock gate opens before the real work ----
            with tc.tile_pool(name="pp", bufs=4, space="PSUM") as pp:
                warm_sb = const.tile([128, 512], BF16, tag="warm")
                nc.vector.memset(warm_sb, 0.0)
                warm_ps = pp.tile([128, 512], F32, tag="pp")
                for _ in range(2):
                    nc.tensor.matmul(
                        out=warm_ps, lhsT=warm_sb[:, 0:128], rhs=warm_sb[:, :],
                        start=True, stop=True,
                    )

                # ---- phase 1: qT/kT [feature, s] = w_qkv^T @ x; bias folded
                #      into the eviction (per-partition, features-major) ----
                for t in range(4):  # feature tiles: q0,q1,k0,k1
                    for nb in range(4):  # s blocks of 512
                        ps = pp.tile([128, 512], F32, tag="pp")
                        for ks in range(2):
                            nc.tensor.matmul(
                                out=ps,
                                lhsT=wqk_sb[:, ks * 512 + t * 128 : ks * 512 + t * 128 + 128],
                                rhs=xt_sb[:, ks * S + nb * 512 : ks * S + nb * 512 + 512],
                                start=(ks == 0), stop=(ks == 1),
                            )
                        dst = (qT_sb if t < 2 else kT_sb)[
                            :, (t % 2) * S + nb * 512 : (t % 2) * S + nb * 512 + 512
                        ]
                        nc.vector.tensor_scalar_add(
                            out=dst, in0=ps, scalar1=bqkc_sb[:, t : t + 1]
                        )

                # ---- phase 2: v (natural layout, padded 64-wide head slots,
                #      ones column at j=32 for rowsums) ----
                for st in range(16):
                    ps = pp.tile([128, 264], F32, tag="ppv")
                    for ks in range(2):
                        nc.tensor.matmul(
                            out=ps,
                            lhsT=xt_sb[:, ks * S + st * 128 : ks * S + st * 128 + 128],
                            rhs=wv_sb[:, ks * 264 : ks * 264 + 264],
                            start=(ks == 0), stop=False,
                        )
                    # bias row also plants the rowsum ones-columns
                    nc.tensor.matmul(
                        out=ps,
                        lhsT=ones1_sb[0:1, 0:128],
                        rhs=bv_sb[0:1, 0:264],
                        start=False, stop=True,
                    )
                    dst = v_sb[:, st * 264 : st * 264 + 264]
                    nc.scalar.copy(out=dst, in_=ps)

            # ---- phase 3: attention ----
            stg_tiles = {}   # (qb, g, h) -> stg tile
            rsg_tiles = {}   # (qb, g) -> packed bf16 rowsum gather tile

            with (
                tc.tile_pool(name="scp", bufs=3, space="PSUM") as scp,
                tc.tile_pool(name="cxp", bufs=2, space="PSUM") as cxp,
            ):
                def emit_ctx(qb, g, kt, ctx_t, eTs):
                    # ctx accumulation for (g, kt): 4 col-mode matmuls
                    for h in range(2):
                        cps = ctx_t[h]
                        eT = eTs[h]
                        vc = kt * 264 + (4 * g + 2 * h) * 33
                        nc.tensor.matmul(
                            out=cps[0:33, :],
                            lhsT=v_sb[:, vc : vc + 33],
                            rhs=eT[:, 0:512],
                            start=(kt == 0), stop=(kt == 15),
                            tile_position=(0, 0), skip_group_check=True,
                        )
                        nc.tensor.matmul(
                            out=cps[64:97, :],
                            lhsT=v_sb[:, vc + 33 : vc + 66],
                            rhs=eT[:, 512:1024],
                            start=(kt == 0), stop=(kt == 15),
                            tile_position=(0, 64), skip_group_check=True,
                        )

                def emit_recip(qb, g):
                    # rowsum reciprocal for the 4 (h,side) rows of (qb,g),
                    # packed [64,32] (free size 32) so the DVE cost is tiny;
                    # result lands in rscr2 rows for the broadcast DMAs
                    rsgb = rsg_tiles.pop((qb, g))
                    rsgf = ev.tile([64, 32], F32, tag="rsgf")
                    nc.vector.tensor_copy(out=rsgf, in_=rsgb)
                    rsr = ev.tile([64, 32], F32, tag="rsr")
                    nc.vector.reciprocal(out=rsr, in_=rsgf)
                    rsb = ev.tile([64, 32], BF16, tag="rsb")
                    nc.vector.tensor_copy(out=rsb, in_=rsr)
                    r0 = qb * 8 + g * 4
                    nc.sync.dma_start(out=rscr2[r0 : r0 + 4, :], in_=rsb)

                def emit_norm(qb, g, tail=False):
                    # normalization muls for the 2 (h) pairs of (qb,g); on
                    # the otherwise-idle GpSimd mid-stream, on the (faster)
                    # DVE for the tail group where nothing else competes
                    for h in range(2):
                        stg = stg_tiles.pop((qb, g, h))
                        rcb = rcbp.tile([128, 512], BF16, tag="rcb",
                                        name=f"rcb_{qb}_{g}_{h}")
                        r0 = qb * 8 + g * 4 + h * 2
                        nc.sync.dma_start(
                            out=rcb[0:32, :],
                            in_=rscr2[r0 : r0 + 1, :].to_broadcast((32, 512)),
                        )
                        nc.sync.dma_start(
                            out=rcb[64:96, :],
                            in_=rscr2[r0 + 1 : r0 + 2, :].to_broadcast((32, 512)),
                        )
                        eng = nc.vector if tail else nc.gpsimd
                        dst = ctxT_sb[2 * g + h]
                        eng.tensor_mul(
                            out=dst[0:32, qb * 512 : qb * 512 + 512],
                            in0=stg[0:32, :], in1=rcb[0:32, :],
                        )
                        eng.tensor_mul(
                            out=dst[64:96, qb * 512 : qb * 512 + 512],
                            in0=stg[64:96, :], in1=rcb[64:96, :],
                        )

                step = 0
                groups = [(qb, g) for qb in range(4) for g in range(2)]
                for gi, (qb, g) in enumerate(groups):
                    ctx_t = [
                        cxp.tile([128, 512], F32, tag="cx",
                                 name=f"cx_{qb}_{g}_{h}")
                        for h in range(2)
                    ]
                    eTs_by_kt = {}
                    for kt in range(16):
                        # deferred normalization work for the previous group
                        # (emitted here so its DMA roundtrip latency never
                        # blocks the in-order engine queues)
                        if gi > 0:
                            if kt == 3:
                                emit_recip(*groups[gi - 1])
                            if kt == 9:
                                emit_norm(*groups[gi - 1])
                        # scores: all 4 row-tiled matmuls of this kt adjacent
                        # so they overlap 4-way on the PE
                        scs = []
                        for h in range(2):
                            sc = scp.tile([128, 1024], F32, tag="sc",
                                          name=f"sc_{qb}_{g}_{kt}_{h}")
                            for jj, j in enumerate((2 * h, 2 * h + 1)):
                                nc.tensor.matmul(
                                    out=sc[:, jj * 512 : jj * 512 + 512],
                                    lhsT=kT_sb[32 * j : 32 * j + 32,
                                               g * S + kt * 128 : g * S + kt * 128 + 128],
                                    rhs=qT_sb[32 * j : 32 * j + 32,
                                              g * S + qb * 512 : g * S + qb * 512 + 512],
                                    start=True, stop=True,
                                    tile_position=(32 * j, 0),
                                )
                            scs.append(sc)
                        eTs = []
                        for h in range(2):
                            eT = etp.tile([128, 1024], BF16, tag="eT")
                            if ACT_PAT[step % 32]:
                                nc.scalar.activation(
                                    out=eT, in_=scs[h], func=EXP, scale=SCALE,
                                )
                            else:
                                nc.vector.tensor_scalar(
                                    out=eT.bitcast(I16), in0=scs[h],
                                    scalar1=EXP_A, scalar2=EXP_B,
                                    op0=mybir.AluOpType.mult,
                                    op1=mybir.AluOpType.add,
                                )
                            eTs.append(eT)
                            step += 1
                        eTs_by_kt[kt] = eTs
                        if kt >= 1:
                            emit_ctx(qb, g, kt - 1, ctx_t,
                                     eTs_by_kt.pop(kt - 1))
                    emit_ctx(qb, g, 15, ctx_t, eTs_by_kt.pop(15))
                    # evict ctx PSUM (bf16 staging, one [97,512] copy per
                    # pair); rowsum rows (32/96, in bf16) are gathered via
                    # tiny SBUF->SBUF DMAs into a packed [64,32] tile for
                    # the reciprocal (row j of 512 -> 16 partitions x 32)
                    rsgb = ev.tile([64, 32], BF16, tag="rsgb",
                                   name=f"rsgb_{qb}_{g}")
                    for h in range(2):
                        stg = stgp.tile([128, 512], BF16, tag="stg",
                                        name=f"stg_{qb}_{g}_{h}")
                        nc.vector.tensor_copy(
                            out=stg[0:97, :], in_=ctx_t[h][0:97, :]
                        )
                        stg_tiles[(qb, g, h)] = stg
                        p0 = h * 32
                        nc.sync.dma_start(out=rsgb[p0 : p0 + 16, :],
                                          in_=stg[32:33, :])
                        nc.sync.dma_start(out=rsgb[p0 + 16 : p0 + 32, :],
                                          in_=stg[96:97, :])
                    rsg_tiles[(qb, g)] = rsgb
                # tail normalization for the last group
                emit_recip(3, 1)
                emit_norm(3, 1, tail=True)

            # ---- phase 4: out = ctxT^T @ w_out_perm + b_out; evictions
            #      alternate ACT/DVE ----
            with tc.tile_pool(name="op", bufs=4, space="PSUM") as op:
                for st in range(16):
                    ps = op.tile([128, 256], F32, tag="op")
                    for kk in range(4):
                        nc.tensor.matmul(
                            out=ps,
                            lhsT=ctxT_sb[kk][:, st * 128 : st * 128 + 128],
                            rhs=wo_sb[:, kk * 256 : kk * 256 + 256],
                            start=(kk == 0), stop=(kk == 3),
                        )
                    # evictions alternate ACT/DVE; the out DMAs are issued
                    # from ACT (HWDGE) and GpSimd (SWDGE) so the tail never
                    # serializes on the sync queue's ~600ns/issue
                    ot = ev.tile([128, 256], F32, tag="ot")
                    if st % 2 == 0:
                        nc.scalar.copy(out=ot, in_=ps)
                        nc.scalar.dma_start(
                            out=out[st * 128 : st * 128 + 128, :], in_=ot
                        )
                    else:
                        nc.vector.tensor_copy(out=ot, in_=ps)
                        nc.sync.dma_start(
                            out=out[st * 128 : st * 128 + 128, :], in_=ot
                        )
    if legalize:
        _legalize_sync_waits(nc)
    return nc


_NC_CACHE = None


def _get_nc():
    global _NC_CACHE
    if _NC_CACHE is None:
        _NC_CACHE = _build_nc()
    return _NC_CACHE


def _ks_layout(a, nk, cols):
    """[nk*128, cols] -> [128, nk*cols] with [p, k*cols+c] = a[k*128+p, c]."""
    return np.ascontiguousarray(
        a.reshape(nk, 128, cols).transpose(1, 0, 2).reshape(128, nk * cols)
    )


def _prep_in_maps(x, w_qkv, b_qkv, w_out, b_out):
    x = np.asarray(x, dtype=np.float32)
    w_qkv = np.asarray(w_qkv, dtype=np.float32)
    b_qkv = np.asarray(b_qkv, dtype=np.float32)
    w_out = np.asarray(w_out, dtype=np.float32)
    b_out = np.asarray(b_out, dtype=np.float32)

    # shared (per-core identical) weight layouts
    wqk_l = _ks_layout(w_qkv[:, : 2 * H], 2, 512).astype(NPBF16)

    # v weights: 64-wide slot per head: [v_h (32) | ones-col | 31 zero]
    # (the ones column itself is planted via the bias matmul; v bias is the
    # spec's b_qkv v-slice)
    wpad = np.zeros((H, 264), np.float32)
    bvr = np.zeros((1, 264), np.float32)
    for h in range(NH):
        c0 = h * 33
        wpad[:, c0 : c0 + 32] = w_qkv[:, 2 * H + h * HD : 2 * H + (h + 1) * HD]
        bvr[0, c0 : c0 + 32] = b_qkv[2 * H + h * HD : 2 * H + (h + 1) * HD]
        bvr[0, c0 + 32] = 1.0  # ones column -> rowsum row
    wv_l = _ks_layout(wpad, 2, 264).astype(NPBF16)

    # w_out rows permuted into the ctxT slot layout (zeros in pad slots)
    wo_perm = np.zeros((512, H), np.float32)
    for pair in range(4):
        for side in range(2):
            h = 2 * pair + side
            r0 = pair * 128 + side * 64
            wo_perm[r0 : r0 + 32, :] = w_out[h * HD : (h + 1) * HD, :]
    wo_perm[32, :] = b_out  # multiplied by the ctxT[0] ones row
    wo_l = _ks_layout(wo_perm, 4, 256).astype(NPBF16)

    shared = {
        "wqk": wqk_l,
        "wv": wv_l,
        "bv": bvr.astype(NPBF16),
        "wo": wo_l,
        "bqkc": np.ascontiguousarray(
            b_qkv[: 2 * H].astype(np.float32).reshape(4, 128).T
        ),
        "ones": np.ones((1, 512), NPBF16),
        "zrow": np.concatenate([np.zeros((1, 2048), NPBF16), np.ones((1, 2048), NPBF16)]),
    }
    in_maps = []
    for b in range(B):
        xt = _ks_layout(np.ascontiguousarray(x[b].T), 2, S).astype(NPBF16)
        in_maps.append({"xt": xt, **shared})
    return in_maps


def kernel(x, w_qkv, b_qkv, w_out, b_out):
    in_maps = _prep_in_maps(x, w_qkv, b_qkv, w_out, b_out)
    nc = _get_nc()
    res = run_bass_kernel_spmd(nc, in_maps, list(range(N_CORES)), **TRACE_OPTS)
    global LAST_RESULT
    LAST_RESULT = res
    return np.stack([res.results[b]["out"] for b in range(B)], axis=0)
